# revision 4
# baseline (speedup 1.0000x reference)
"""Trainium2 Bass kernel for nn_Diffusion (B=8192, D=2048, T=1000).

Data-parallel over 8 NeuronCores (1024 batch rows each). Per core:
  Phase 0: C = (Wq^T @ Wk) * D^-0.5  (float32r matmuls) -> DRAM scratch
  Phase 1: per 128-row chunk: gather LN(t_emb)/sqrt_ac/sqrt_omac by t,
           x_noisy, LayerNorms, PE-transpose F, P^T = C^T F^T, logits via PE,
           diag-extract, softmax -> w, g = w @ F (in transposed layout) -> DRAM
  Phase 2: Wv^T built by PE transpose; pred = g @ Wv^T; loss partials.
Host: builds the constant LN(t_emb) table, shards, sums loss partials.
"""

import math
import numpy as np
from contextlib import ExitStack

import jax
from jax.experimental.shard_map import shard_map
from jax.sharding import Mesh, PartitionSpec

import concourse.bass as bass
import concourse.bacc as bacc
import concourse.tile as tile
from concourse import mybir
from concourse.bass2jax import (
    _bass_exec_p,
    partition_id_tensor,
    install_neuronx_cc_hook,
)
from concourse.masks import make_identity

F32 = mybir.dt.float32
F32R = mybir.dt.float32r
I32 = mybir.dt.int32
AF = mybir.ActivationFunctionType
OP = mybir.AluOpType

B, D, T = 8192, 2048, 1000
N_CORES = 8
BC = B // N_CORES          # rows per core
NCHUNK = BC // 128         # 8 chunks of 128 rows
TBL_W = 2056               # 2048 temb + sqrt_ac + sqrt_omac + pad


def _layernorm_inplace(nc, small, x, tag_prefix=""):
    """x: [128, 2048] f32 SBUF AP, normalized in place over free dim."""
    st = small.tile([128, 4, 6], F32, tag="ln_st")
    xg = x.rearrange("p (g d) -> p g d", g=4)
    for g in range(4):
        nc.vector.bn_stats(out=st[:, g, :], in_=xg[:, g, :])
    mv = small.tile([128, 2], F32, tag="ln_mv")
    nc.vector.bn_aggr(out=mv[:], in_=st[:])
    veps = small.tile([128, 1], F32, tag="ln_veps")
    nc.vector.tensor_scalar_add(veps[:], mv[:, 1:2], 1e-5)
    std = small.tile([128, 1], F32, tag="ln_std")
    nc.scalar.activation(std[:], veps[:], AF.Sqrt)
    inv = small.tile([128, 1], F32, tag="ln_inv")
    nc.vector.reciprocal(inv[:], std[:])
    # one Newton step in rsqrt space: inv *= (1.5 - 0.5*veps*inv^2)
    t1 = small.tile([128, 1], F32, tag="ln_t1")
    nc.scalar.activation(t1[:], inv[:], AF.Square)
    nc.vector.tensor_mul(t1[:], t1[:], veps[:])
    nc.vector.tensor_scalar(t1[:], t1[:], -0.5, 1.5, OP.mult, OP.add)
    nc.vector.tensor_mul(inv[:], inv[:], t1[:])
    nc.vector.tensor_scalar(x, x, mv[:, 0:1], inv[:], OP.subtract, OP.mult)


def build_nc():
    nc = bacc.Bacc("TRN2", target_bir_lowering=False, debug=False)

    xs_in = nc.dram_tensor("x_start", [BC, D], F32, kind="ExternalInput")
    nz_in = nc.dram_tensor("noise", [BC, D], F32, kind="ExternalInput")
    cs_in = nc.dram_tensor("cond_src", [BC, D], F32, kind="ExternalInput")
    ct_in = nc.dram_tensor("cond_tgt", [BC, D], F32, kind="ExternalInput")
    t_in = nc.dram_tensor("t", [BC, 1], I32, kind="ExternalInput")
    wq_in = nc.dram_tensor("w_q", [D, D], F32R, kind="ExternalInput")
    wk_in = nc.dram_tensor("w_k", [D, D], F32R, kind="ExternalInput")
    wv_in = nc.dram_tensor("w_v", [D, D], F32, kind="ExternalInput")
    tbl_in = nc.dram_tensor("tbl", [T, TBL_W], F32, kind="ExternalInput")

    pred_out = nc.dram_tensor("pred", [BC, D], F32, kind="ExternalOutput")
    loss_out = nc.dram_tensor("lossp", [128, NCHUNK * 4], F32, kind="ExternalOutput")

    with tile.TileContext(nc) as tc, ExitStack() as ctx:
        dram = ctx.enter_context(tc.tile_pool(name="dram", bufs=1, space="DRAM"))
        c_dram = dram.tile([D, D], F32R)
        g_dram = dram.tile([NCHUNK, 128, D], F32R)
        w_dram = dram.tile([NCHUNK, 4, 128], F32)

        consts = ctx.enter_context(tc.tile_pool(name="consts", bufs=1))
        ident = consts.tile([128, 128], F32)
        make_identity(nc, ident[:])
        # diag mask [128, 4, 128]: 1 where inner index == partition
        maskt = consts.tile([128, 4, 128], F32)
        nc.gpsimd.memset(maskt[:], 0.0)
        nc.gpsimd.affine_select(
            out=maskt[:], in_=maskt[:], compare_op=OP.not_equal, fill=1.0,
            base=0, pattern=[[0, 4], [-1, 128]], channel_multiplier=1,
        )
        loss_sb = consts.tile([128, NCHUNK * 4], F32)
        nc.vector.memset(loss_sb[:], 0.0)

        # ---------------- Phase 0: C = (Wq^T Wk) * D^-0.5 ----------------
        with (
            tc.tile_pool(name="ph0", bufs=1) as ph0,
            tc.tile_pool(name="ph0s", bufs=2) as ph0s,
            tc.tile_pool(name="ps0", bufs=4, space="PSUM") as ps0,
        ):
            wk_sb = ph0.tile([128, 16, D], F32R)
            nc.sync.dma_start(
                out=wk_sb[:], in_=wk_in[:].rearrange("(t p) d -> p t d", p=128))
            wq_r = wq_in[:].rearrange("(t p) d -> p t d", p=128)
            for m in range(16):
                wqm = ph0s.tile([128, 16, 128], F32R, tag="wqm")
                nc.sync.dma_start(out=wqm[:], in_=wq_r[:, :, m * 128:(m + 1) * 128])
                cbuf = ph0s.tile([128, D], F32R, tag="cbuf")
                for n in range(4):
                    ps = ps0.tile([128, 512], F32)
                    for e in range(16):
                        nc.tensor.matmul(
                            out=ps[:], lhsT=wqm[:, e, :],
                            rhs=wk_sb[:, e, n * 512:(n + 1) * 512],
                            start=(e == 0), stop=(e == 15),
                        )
                    nc.scalar.activation(
                        out=cbuf[:, n * 512:(n + 1) * 512], in_=ps[:],
                        func=AF.Copy, scale=float(D) ** -0.5)
                nc.sync.dma_start(out=c_dram[m * 128:(m + 1) * 128, :], in_=cbuf[:])

        # ---------------- Phase 1 ----------------
        with (
            tc.tile_pool(name="cpool", bufs=1) as cpool,
            tc.tile_pool(name="ftpool", bufs=1) as ftpool,
            tc.tile_pool(name="slab", bufs=3) as slab,
            tc.tile_pool(name="gpool", bufs=2) as gpool,
            tc.tile_pool(name="pjp", bufs=3) as pjp,
            tc.tile_pool(name="small", bufs=2) as small,
            tc.tile_pool(name="med", bufs=2) as med,
            tc.tile_pool(name="wrepp", bufs=1) as wrepp,
            tc.tile_pool(name="ps_tr", bufs=2, space="PSUM") as ps_tr,
            tc.tile_pool(name="ps_p", bufs=2, space="PSUM") as ps_p,
            tc.tile_pool(name="ps_l", bufs=4, space="PSUM") as ps_l,
        ):
            C_sb = cpool.tile([128, 16, D], F32R)
            nc.sync.dma_start(
                out=C_sb[:], in_=c_dram[:].rearrange("(t p) d -> p t d", p=128))

            for c in range(NCHUNK):
                r0, r1 = c * 128, (c + 1) * 128
                tsb = small.tile([128, 1], I32, tag="tsb")
                nc.sync.dma_start(out=tsb[:], in_=t_in[r0:r1, :])
                gath = slab.tile([128, TBL_W], F32, tag="slab")
                nc.gpsimd.indirect_dma_start(
                    out=gath[:], out_offset=None, in_=tbl_in[:],
                    in_offset=bass.IndirectOffsetOnAxis(ap=tsb[:, :1], axis=0),
                )
                xs = slab.tile([128, TBL_W], F32, tag="slab")
                nc.sync.dma_start(out=xs[:, :D], in_=xs_in[r0:r1, :])
                nc.vector.tensor_scalar_mul(xs[:, :D], xs[:, :D], gath[:, 2048:2049])
                nz = slab.tile([128, TBL_W], F32, tag="slab")
                nc.sync.dma_start(out=nz[:, :D], in_=nz_in[r0:r1, :])
                nc.vector.tensor_scalar_mul(nz[:, :D], nz[:, :D], gath[:, 2049:2050])
                nc.vector.tensor_add(xs[:, :D], xs[:, :D], nz[:, :D])
                _layernorm_inplace(nc, small, xs[:, :D])

                FT = ftpool.tile([128, 16, 512], F32R)

                def emit_transposes(src_ap, s):
                    for k4 in range(4):
                        pt = ps_tr.tile([128, 4, 128], F32, tag="pt")
                        for kk in range(4):
                            k = k4 * 4 + kk
                            nc.tensor.transpose(
                                out=pt[:, kk, :],
                                in_=src_ap[:, k * 128:(k + 1) * 128],
                                identity=ident[:],
                            )
                        nc.scalar.copy(
                            out=FT[:, k4 * 4:k4 * 4 + 4, s * 128:(s + 1) * 128],
                            in_=pt[:],
                        )

                # token order: 0=x_noisy, 1=t_emb(LN'd), 2=cond_src, 3=cond_tgt
                emit_transposes(xs[:, :D], 0)
                emit_transposes(gath[:, :D], 1)
                for s, src in ((2, cs_in), (3, ct_in)):
                    cd = slab.tile([128, TBL_W], F32, tag="slab")
                    nc.sync.dma_start(out=cd[:, :D], in_=src[r0:r1, :])
                    _layernorm_inplace(nc, small, cd[:, :D])
                    emit_transposes(cd[:, :D], s)

                # P^T tiles + logits accumulation
                psL = [ps_l.tile([128, 512], F32, tag="psl", name=f"psL{qs}")
                       for qs in range(4)]
                for j in range(16):
                    pp = ps_p.tile([128, 512], F32, tag="pp")
                    for k in range(16):
                        nc.tensor.matmul(
                            out=pp[:], lhsT=C_sb[:, k, j * 128:(j + 1) * 128],
                            rhs=FT[:, k, :], start=(k == 0), stop=(k == 15),
                        )
                    pj = pjp.tile([128, 512], F32R, tag="pj")
                    nc.scalar.copy(out=pj[:], in_=pp[:])
                    for qs in range(4):
                        nc.tensor.matmul(
                            out=psL[qs][:], lhsT=pj[:, qs * 128:(qs + 1) * 128],
                            rhs=FT[:, j, :], start=(j == 0), stop=(j == 15),
                        )

                # diagonal extraction -> logits [128, 16] (qs-major)
                Lt = small.tile([128, 16], F32, tag="Lt")
                for qs in range(4):
                    dg = med.tile([128, 512], F32, tag="dg")
                    nc.vector.tensor_mul(dg[:], psL[qs][:], maskt[:].rearrange("p a b -> p (a b)"))
                    nc.vector.reduce_sum(
                        out=Lt[:, qs * 4:(qs + 1) * 4],
                        in_=dg[:].rearrange("p (k b) -> p k b", b=128),
                        axis=mybir.AxisListType.X,
                    )
                # softmax (no max-sub needed; logits are O(1)) and w[ks]
                et = small.tile([128, 16], F32, tag="et")
                nc.scalar.activation(et[:], Lt[:], AF.Exp)
                es = small.tile([128, 4], F32, tag="es")
                nc.vector.reduce_sum(
                    out=es[:], in_=et[:].rearrange("p (q k) -> p q k", k=4),
                    axis=mybir.AxisListType.X)
                rr = small.tile([128, 4], F32, tag="rr")
                nc.vector.reciprocal(rr[:], es[:])
                nc.vector.tensor_scalar_mul(rr[:], rr[:], 0.25)
                w4 = small.tile([128, 4], F32, tag="w4")
                nc.vector.tensor_scalar_mul(w4[:], et[:, 0:4], rr[:, 0:1])
                for q in range(1, 4):
                    tq = small.tile([128, 4], F32, tag="tq")
                    nc.vector.tensor_scalar_mul(tq[:], et[:, q * 4:(q + 1) * 4], rr[:, q:q + 1])
                    nc.vector.tensor_add(w4[:], w4[:], tq[:])

                # broadcast w across partitions via DRAM roundtrip
                pw = ps_tr.tile([4, 128], F32, tag="pt")
                nc.tensor.transpose(out=pw[:], in_=w4[:], identity=ident[:])
                wrow = small.tile([4, 128], F32, tag="wrow")
                nc.scalar.copy(out=wrow[:], in_=pw[:])
                nc.sync.dma_start(out=w_dram[c], in_=wrow[:])
                wrep = wrepp.tile([128, 4, 128], F32, tag="wrep")
                wsrc = w_dram[c]  # [4, 128] dram AP
                wb = bass.AP(
                    tensor=wsrc.tensor, offset=wsrc.offset,
                    ap=[[0, 128]] + list(wsrc.ap),
                )
                nc.sync.dma_start(out=wrep[:], in_=wb)

                # g^T tiles: gt_k[p, b] = sum_s FT_k[p, s*128+b] * w[b, s]
                for k in range(16):
                    gtmp = med.tile([128, 512], F32, tag="gtmp")
                    nc.vector.tensor_mul(
                        gtmp[:],
                        FT[:, k, :].bitcast(F32),
                        wrep[:].rearrange("p a b -> p (a b)"),
                    )
                    gt = small.tile([128, 128], F32, tag="gt")
                    gin = bass.AP(
                        tensor=gtmp[:].tensor, offset=gtmp[:].offset,
                        ap=[gtmp[:].ap[0], [1, 128], [128, 4]],
                    )
                    nc.vector.reduce_sum(out=gt[:], in_=gin, axis=mybir.AxisListType.X)
                    gtr = gpool.tile([128, 128], F32R, tag="gtr")
                    nc.scalar.copy(out=gtr[:], in_=gt[:])
                    nc.sync.dma_start(
                        out=g_dram[c, :, k * 128:(k + 1) * 128], in_=gtr[:])

        # ---------------- Phase 2: pred = g @ Wv^T, loss ----------------
        with (
            tc.tile_pool(name="wvt", bufs=1) as wvtp,
            tc.tile_pool(name="ph2s", bufs=2) as ph2s,
            tc.tile_pool(name="ps_tr2", bufs=2, space="PSUM") as ps_tr2,
            tc.tile_pool(name="ps_pred", bufs=4, space="PSUM") as ps_pred,
        ):
            WvT = wvtp.tile([128, 16, D], F32R)
            wv_r = wv_in[:].rearrange("(t p) d -> p t d", p=128)
            for k in range(16):
                wvcol = ph2s.tile([128, 16, 128], F32, tag="wvcol")
                nc.sync.dma_start(out=wvcol[:], in_=wv_r[:, :, k * 128:(k + 1) * 128])
                for j4 in range(4):
                    pt = ps_tr2.tile([128, 4, 128], F32, tag="pt2")
                    for jj in range(4):
                        nc.tensor.transpose(
                            out=pt[:, jj, :], in_=wvcol[:, j4 * 4 + jj, :],
                            identity=ident[:])
                    nc.scalar.copy(
                        out=WvT[:, k, j4 * 512:(j4 + 1) * 512].rearrange(
                            "p (a b) -> p a b", b=128),
                        in_=pt[:])

            for c in range(NCHUNK):
                r0, r1 = c * 128, (c + 1) * 128
                gt_c = ph2s.tile([128, D], F32R, tag="gt_c")
                nc.sync.dma_start(out=gt_c[:], in_=g_dram[c])
                xs2 = ph2s.tile([128, D], F32, tag="xs2")
                nc.sync.dma_start(out=xs2[:], in_=xs_in[r0:r1, :])
                for n in range(4):
                    pp = ps_pred.tile([128, 512], F32, tag="ppred")
                    for k in range(16):
                        nc.tensor.matmul(
                            out=pp[:], lhsT=gt_c[:, k * 128:(k + 1) * 128],
                            rhs=WvT[:, k, n * 512:(n + 1) * 512],
                            start=(k == 0), stop=(k == 15),
                        )
                    predsb = ph2s.tile([128, 512], F32, tag="predsb")
                    nc.scalar.copy(out=predsb[:], in_=pp[:])
                    nc.sync.dma_start(
                        out=pred_out[r0:r1, n * 512:(n + 1) * 512], in_=predsb[:])
                    dt = ph2s.tile([128, 512], F32, tag="dt")
                    nc.vector.tensor_tensor(
                        out=dt[:], in0=predsb[:], in1=xs2[:, n * 512:(n + 1) * 512],
                        op=OP.subtract)
                    nc.scalar.activation(
                        out=dt[:], in_=dt[:], func=AF.Square,
                        accum_out=loss_sb[:, c * 4 + n:c * 4 + n + 1])
            nc.sync.dma_start(out=loss_out[:], in_=loss_sb[:])

    nc.compile()
    return nc


def make_tbl(sqrt_ac, sqrt_omac):
    """Constant table: row t = [LN(t_emb(t)) (2048), sqrt_ac[t], sqrt_omac[t], 0...]."""
    half = D // 2
    freqs = np.exp(
        np.arange(half, dtype=np.float32) * (-math.log(10000.0) / (half - 1)))
    ts = np.arange(T, dtype=np.float32)
    emb = (ts[:, None] * freqs[None, :]).astype(np.float32)
    e64 = emb.astype(np.float64)
    temb = np.concatenate([np.sin(e64), np.cos(e64)], axis=1)  # [T, D] f64
    mu = temb.mean(axis=1, keepdims=True)
    var = ((temb - mu) ** 2).mean(axis=1, keepdims=True)
    ln = (temb - mu) / np.sqrt(var + 1e-5)
    tbl = np.zeros((T, TBL_W), dtype=np.float32)
    tbl[:, :D] = ln.astype(np.float32)
    tbl[:, 2048] = np.asarray(sqrt_ac, dtype=np.float32)
    tbl[:, 2049] = np.asarray(sqrt_omac, dtype=np.float32)
    return tbl


_CACHE = {}


def _get_compiled():
    if "ck" not in _CACHE:
        install_neuronx_cc_hook()
        nc = build_nc()
        # build jitted shard_map callable over 8 cores
        in_names, out_names, out_avals = [], [], []
        partition_name = nc.partition_id_tensor.name if nc.partition_id_tensor else None
        for alloc in nc.m.functions[0].allocations:
            if not isinstance(alloc, mybir.MemoryLocationSet):
                continue
            name = alloc.memorylocations[0].name
            if alloc.kind == "ExternalInput":
                if name != partition_name:
                    in_names.append(name)
            elif alloc.kind == "ExternalOutput":
                out_names.append(name)
                out_avals.append(jax.core.ShapedArray(
                    tuple(alloc.tensor_shape), mybir.dt.np(alloc.dtype)))
        all_in = list(in_names) + list(out_names)
        if partition_name is not None:
            all_in.append(partition_name)

        def _body(*args):
            operands = list(args)
            if partition_name is not None:
                operands.append(partition_id_tensor())
            return tuple(_bass_exec_p.bind(
                *operands,
                out_avals=tuple(out_avals), in_names=tuple(all_in),
                out_names=tuple(out_names), lowering_input_output_aliases=(),
                sim_require_finite=True, sim_require_nnan=True, nc=nc,
            ))

        devices = jax.devices()[:N_CORES]
        mesh = Mesh(np.asarray(devices), ("core",))
        nin = len(in_names)
        nout = len(out_names)
        fn = jax.jit(
            shard_map(_body, mesh=mesh,
                      in_specs=(PartitionSpec("core"),) * (nin + nout),
                      out_specs=(PartitionSpec("core"),) * nout,
                      check_rep=False),
            keep_unused=True,
        )
        _CACHE["ck"] = (fn, in_names, out_names, out_avals, mesh)
    return _CACHE["ck"]


def kernel(x_start, t, noise, cond_src, cond_tgt, w_q, w_k, w_v, sqrt_ac, sqrt_omac):
    fn, in_names, out_names, out_avals, mesh = _get_compiled()

    x_start = np.asarray(x_start, dtype=np.float32)
    noise = np.asarray(noise, dtype=np.float32)
    cond_src = np.asarray(cond_src, dtype=np.float32)
    cond_tgt = np.asarray(cond_tgt, dtype=np.float32)
    t_np = np.asarray(t, dtype=np.int32).reshape(B, 1)
    w_q = np.ascontiguousarray(np.asarray(w_q, dtype=np.float32))
    w_k = np.ascontiguousarray(np.asarray(w_k, dtype=np.float32))
    w_v = np.ascontiguousarray(np.asarray(w_v, dtype=np.float32))
    tbl = make_tbl(sqrt_ac, sqrt_omac)

    per_core = {
        "x_start": lambda c: x_start[c * BC:(c + 1) * BC],
        "noise": lambda c: noise[c * BC:(c + 1) * BC],
        "cond_src": lambda c: cond_src[c * BC:(c + 1) * BC],
        "cond_tgt": lambda c: cond_tgt[c * BC:(c + 1) * BC],
        "t": lambda c: t_np[c * BC:(c + 1) * BC],
        "w_q": lambda c: w_q,
        "w_k": lambda c: w_k,
        "w_v": lambda c: w_v,
        "tbl": lambda c: tbl,
    }
    concat_in = [
        np.ascontiguousarray(np.concatenate(
            [per_core[nm](c) for c in range(N_CORES)], axis=0))
        for nm in in_names
    ]
    concat_zeros = [
        np.zeros((N_CORES * av.shape[0], *av.shape[1:]), av.dtype)
        for av in out_avals
    ]
    outs = fn(*concat_in, *concat_zeros)
    outs = [np.asarray(o) for o in outs]
    res = {nm: outs[i] for i, nm in enumerate(out_names)}

    pred = res["pred"].reshape(B, D)  # cores concatenated on axis 0 == batch
    lossp = res["lossp"].astype(np.float64)
    loss = np.float32(lossp.sum() / (B * D))
    return np.array(loss, dtype=np.float32), pred
